# revision 1
# baseline (speedup 1.0000x reference)
"""LocallyConnected1d Bass kernel for 8 TRN2 NeuronCores.

Problem: x [64, 64, 512] f32, weight [1, 64, 64, 504, 9] f32
         out[b, o, l] = sum_{i,k} x[b, i, l+k] * weight[0, o, i, l, k]

Strategy:
  - Shard L_out=504 across 8 cores (63 positions each); x gets a 71-col halo.
  - Per position l the contraction is (i, k) = 576 wide with its own weight
    matrix. Split k into 4 pairs + 1 single (zero-padded to a uniform 5 slots
    of 128 contraction rows).
  - Stationary operand = x-column pair tile [128, 64]: rows 0-63 = x[:, :, p]
    transposed to (i, b), rows 64-127 = x[:, :, p+1]. The moving operand is
    the per-position weight slot [128, 64 (c_out)]. Each position accumulates
    5 matmuls in PSUM (out[b, o]).
  - Even/odd positions write PSUM partitions 0-63 / 64-127 -> PE column
    groups 0/1 run concurrently.
  - Inputs are pre-transposed/padded on host so every DMA is contiguous.
"""

import numpy as np
import ml_dtypes

B = 64
CI = 64
CO = 64
K = 9
L = 512
L_OUT = 504
N_CORES = 8
LP = L_OUT // N_CORES          # 63 positions per core
HALO = LP + K - 1              # 71 x-columns per core
NSLOT = 5                      # 4 k-pairs + 1 single (zero-padded)
NGRP = (LP + 1) // 2           # 32 psum groups of 2 positions
# weight DMA chunks: exactly 8 HWDGE DMAs total (x2 + 5 chunks + 2 out)
# so the 8 DMA semaphore lanes are never reused (a reused lane adds a
# second wait to the out-DMA, exceeding the 1-wait ISA limit).
CHUNK_STARTS = [0, 21, 42]
CHUNK_ENDS = [21, 42, 63]
NWCHUNK = len(CHUNK_STARTS)
CHUNK_OF = [0] * LP
for _c, (_s, _e) in enumerate(zip(CHUNK_STARTS, CHUNK_ENDS)):
    for _l in range(_s, _e):
        CHUNK_OF[_l] = _c

USE_BF16 = True


def _build_bass():
    import concourse.bass as bass
    import concourse.mybir as mybir
    from concourse.tile import TileContext

    dt = mybir.dt.bfloat16 if USE_BF16 else mybir.dt.float32
    nc = bass.Bass()

    x2_d = nc.dram_tensor("x2", [128, HALO * B], dt, kind="ExternalInput")
    wp_d = nc.dram_tensor("wp", [128, LP * NSLOT * CO], dt, kind="ExternalInput")
    out_d = nc.dram_tensor("out", [128, NGRP * CO], mybir.dt.float32,
                           kind="ExternalOutput")

    with TileContext(nc) as tc:
        with (
            tc.tile_pool(name="xc", bufs=1) as xpool,
            tc.tile_pool(name="wc", bufs=NWCHUNK) as wpool,
            tc.tile_pool(name="ps", bufs=1, space="PSUM") as ppool,
            tc.tile_pool(name="ob", bufs=1) as opool,
        ):
            x2 = xpool.tile([128, HALO * B], dt)
            nc.sync.dma_start(out=x2, in_=x2_d[:, :])

            wtiles = []
            for c in range(NWCHUNK):
                lo = CHUNK_STARTS[c] * NSLOT * CO
                hi = CHUNK_ENDS[c] * NSLOT * CO
                t = wpool.tile([128, hi - lo], dt, name=f"wt{c}", tag="wt")
                nc.sync.dma_start(out=t, in_=wp_d[:, lo:hi])
                wtiles.append(t)

            out_sb = opool.tile([128, NGRP * CO], mybir.dt.float32)
            # l=63 does not exist: zero the never-written odd half of the
            # last column group so the out-DMA reads initialized memory.
            nc.vector.memset(
                out_sb[64:128, (NGRP - 1) * CO:NGRP * CO], 0.0)

            # 8 static single-bank PSUM tiles. Position l uses bank l%8,
            # partition half l%2 (-> PE column group l%2, so consecutive
            # position bursts overlap in the array). A position's 5 matmuls
            # run back-to-back: only one accumulation group per PSUM bank
            # "zero region" is ever open (HW constraint), and concurrent
            # copy/matmul never touch the same bank (P10 hazard).
            ptile = [ppool.tile([128, CO], mybir.dt.float32, name=f"pb{t}")
                     for t in range(8)]

            for l in range(LP):
                c = CHUNK_OF[l]
                if l == CHUNK_STARTS[c] and c > 0:
                    # dummy ldweights absorbs the weight-chunk DMA wait so
                    # the following matmul keeps <=1 wait (ISA limit).
                    nc.tensor.ldweights(weights=wtiles[c][:, 0:CO])
                half = l % 2
                outp = ptile[l % 8][half * 64:half * 64 + 64, :]
                for s in range(NSLOT):
                    q = l + 2 * s                 # stationary x column
                    lhsT = x2[:, q * B:(q + 1) * B]
                    off = ((l - CHUNK_STARTS[c]) * NSLOT + s) * CO
                    rhs = wtiles[c][:, off:off + CO]
                    nc.tensor.matmul(outp, lhsT, rhs,
                                     start=(s == 0), stop=(s == NSLOT - 1))
                nc.vector.tensor_copy(
                    out=out_sb[half * 64:half * 64 + 64,
                               (l // 2) * CO:(l // 2 + 1) * CO],
                    in_=outp)
                if l == 31:
                    nc.sync.dma_start(
                        out=out_d[:, :NGRP // 2 * CO],
                        in_=out_sb[:, :NGRP // 2 * CO])
                elif l == LP - 1:
                    nc.sync.dma_start(
                        out=out_d[:, NGRP // 2 * CO:],
                        in_=out_sb[:, NGRP // 2 * CO:])
    _split_multi_waits(nc, mybir)
    return nc


def _split_multi_waits(nc, mybir):
    """This walrus build encodes at most ONE sync wait per instruction.

    Tile attaches multi-wait lists (e.g. on the kernel-tail Drain). Hoist
    all but the last wait onto single-wait NoOps inserted just before the
    instruction on the same engine -- semantically identical (the engine
    stalls at the NoOps instead of at the instruction itself).
    """
    for f in nc.m.functions:
        for bb in f.blocks:
            out = []
            for inst in bb.instructions:
                si = inst.sync_info
                waits = list(si.on_wait) if si is not None and si.on_wait else []
                if len(waits) > 1:
                    for k, w in enumerate(waits[:-1]):
                        out.append(mybir.InstNoOp(
                            name=f"{inst.name}-wsplit{k}",
                            engine=inst.engine,
                            sync_info=mybir.SyncInfo(on_wait=[w], on_update=[]),
                            bass_nofuse=True))
                    inst.sync_info = mybir.SyncInfo(
                        on_wait=[waits[-1]],
                        on_update=list(si.on_update) if si.on_update else [])
                out.append(inst)
            bb.instructions = out


def _prep_inputs(x, weight):
    """Returns list of 8 per-core input dicts."""
    npdt = ml_dtypes.bfloat16 if USE_BF16 else np.float32
    x = np.asarray(x, np.float32)
    w0 = np.asarray(weight, np.float32)[0]        # [CO, CI, L_OUT, K]

    wt = np.ascontiguousarray(w0.transpose(2, 3, 1, 0))   # [L_OUT, K, CI, CO]
    wslots = np.zeros((L_OUT, NSLOT, 128, CO), np.float32)
    wslots[:, :4] = wt[:, :8].reshape(L_OUT, 4, 128, CO)
    wslots[:, 4, :CI] = wt[:, 8]

    xt = x.transpose(1, 2, 0)                     # [CI, L, B]

    in_maps = []
    for m in range(N_CORES):
        hs = LP * m
        x2 = np.zeros((128, HALO, B), np.float32)
        x2[:CI] = xt[:, hs:hs + HALO]
        x2[CI:, :HALO - 1] = xt[:, hs + 1:hs + HALO]
        wp = wslots[hs:hs + LP].transpose(2, 0, 1, 3)     # [128, LP, NSLOT, CO]
        in_maps.append({
            "x2": np.ascontiguousarray(x2.reshape(128, HALO * B)).astype(npdt),
            "wp": np.ascontiguousarray(wp).reshape(128, LP * NSLOT * CO).astype(npdt),
        })
    return in_maps


def _decode_outputs(results):
    outs = []
    for r in results:
        v = np.asarray(r["out"], np.float32).reshape(2, 64, NGRP, CO)
        # v[h, b, g, o] -> out[b, o, l], l = 2g + h
        t = v.transpose(1, 3, 2, 0).reshape(B, CO, NGRP * 2)[:, :, :LP]
        outs.append(t)
    return np.concatenate(outs, axis=2)           # [B, CO, L_OUT]


_CACHED_NC = None


def kernel(x, weight):
    global _CACHED_NC
    from concourse.bass_utils import run_bass_kernel_spmd

    if _CACHED_NC is None:
        _CACHED_NC = _build_bass()
    in_maps = _prep_inputs(x, weight)
    res = run_bass_kernel_spmd(_CACHED_NC, in_maps, core_ids=list(range(N_CORES)))
    return _decode_outputs(res.results)



# revision 12
# speedup vs baseline: 71745.7598x; 71745.7598x over previous
"""LocallyConnected1d Bass kernel for 8 TRN2 NeuronCores.

Problem: x [64, 64, 512] f32, weight [1, 64, 64, 504, 9] f32
         out[b, o, l] = sum_{i,k} x[b, i, l+k] * weight[0, o, i, l, k]

Strategy:
  - Shard L_out=504 across 8 cores (63 positions each); x gets a 71-col halo.
  - Per position pair (l, l+1): 4 pair-slots for l (k=0..7), 4 for l+1
    (k=1..8), and one merged single slot (rows 0-63 = w[l, k=8], rows
    64-127 = w[l+1, k=0]) consumed by two contraction-64 matmuls. 4.5
    weight slots/position -- no zero padding in the weight stream.
  - Every stationary tile is an EVEN-q x-column pair, so x is packed once
    into 128 partitions: block j = [x_{2j} (rows 0-63); x_{2j+1} (rows
    64-127)], transposed to (i, b). 4.6KB/partition, no duplication.
  - Weights travel as fp8 e3m4 (x stays bf16; PSUM accumulates f32), which
    halves the dominant DMA stream. Output staged as bf16, upcast on host.
  - Even/odd positions write PSUM partitions 0-63 / 64-127 (PE column
    groups 0/1 run concurrently), cycling 8 PSUM banks.
  - DMA issue costs ~1.7us of engine time each, so the stream uses few,
    large chunks split across the SP and Activation queues. Position 62's
    weights ride an early Activation DMA and it is computed mid-stream, so
    the final out-DMA covers only positions 56-61 and the post-stream tail
    is one short chain.
"""

import numpy as np
import ml_dtypes

B = 64
CI = 64
CO = 64
K = 9
L = 512
L_OUT = 504
N_CORES = 8
LP = L_OUT // N_CORES          # 63 positions per core
HALO = LP + K - 1              # 71 x-columns per core
NXBLK = (HALO + 1) // 2        # 36 packed x column-pair blocks
NPAIR = LP // 2                # 31 full position pairs (+ lone l=62)
NSLOT = 9 * NPAIR + 5          # 284 weight slots of [128, 64]
NGRP = (LP + 1) // 2           # 32 output column groups

X_SPLIT = 7                    # x blocks 0..6 cover positions 0..5
# SP weight chunks in position-pairs (graduated so the PE, which starts at
# a low clock p-state, never waits at a chunk boundary); the lone position
# 62 rides an early Activation-queue DMA.
W_PAIR_CHUNKS = [4, 4, 6, 8, 9]
assert sum(W_PAIR_CHUNKS) == NPAIR

# compute order: 0..55, then 62 (weights arrive early), then 56..61 so the
# final out-DMA chain is short and overlaps the stream
ORDER = list(range(56)) + [62] + list(range(56, 62))
# out chunks: (gate position, [groups])
OUT_CHUNKS = [
    (41, list(range(0, 21))),
    (55, list(range(21, 28))),
    (62, [31]),
    (61, list(range(28, 31))),
]

WDT_NAME = "float8e3"          # weight dtype: "float8e3" or "bfloat16"


def _build_bass():
    import concourse.bass as bass
    import concourse.mybir as mybir
    from concourse.tile import TileContext

    bf16 = mybir.dt.bfloat16
    wdt = getattr(mybir.dt, WDT_NAME)
    nc = bass.Bass()

    xp_d = nc.dram_tensor("xp", [128, NXBLK * B], bf16, kind="ExternalInput")
    wp_d = nc.dram_tensor("wp", [128, NSLOT * CO], wdt, kind="ExternalInput")
    out_d = nc.dram_tensor("out", [128, NGRP * CO], bf16, kind="ExternalOutput")

    # slot index of position l's s-th slot (s=0..4; s=4 is the merged single)
    def slot_of(l, s):
        pair, odd = divmod(l, 2)
        if pair == NPAIR:              # lone position 62
            return 9 * NPAIR + (s if s < 4 else 4)
        if not odd:
            return 9 * pair + (s if s < 4 else 4)
        return 9 * pair + (5 + s if s < 4 else 4)

    # SP chunk boundaries in slots + the Activation chunk for position 62
    w_chunk_slots = []
    p = 0
    for npairs in W_PAIR_CHUNKS:
        w_chunk_slots.append((9 * p, 9 * (p + npairs)))
        p += npairs
    w62_slots = (9 * NPAIR, NSLOT)
    chunk_of_pos = []
    for l in range(LP):
        hi = max(slot_of(l, s) for s in range(5))
        c = next((i for i, (lo, chi) in enumerate(w_chunk_slots) if hi < chi),
                 len(w_chunk_slots))   # len() -> the w62 chunk
        chunk_of_pos.append(c)

    with TileContext(nc) as tc:
        with (
            tc.tile_pool(name="xc", bufs=2) as xpool,
            tc.tile_pool(name="wc", bufs=len(w_chunk_slots) + 1) as wpool,
            tc.tile_pool(name="ps", bufs=1, space="PSUM") as ppool,
            tc.tile_pool(name="ob", bufs=len(OUT_CHUNKS)) as opool,
        ):
            xa = xpool.tile([128, X_SPLIT * B], bf16, name="xa", tag="xp")
            xb = xpool.tile([128, (NXBLK - X_SPLIT) * B], bf16, name="xb",
                            tag="xp")
            nc.sync.dma_start(out=xa, in_=xp_d[:, :X_SPLIT * B])

            # position 62's slots: early, on the Activation queue (no wait)
            w62 = wpool.tile([128, (w62_slots[1] - w62_slots[0]) * CO], wdt,
                             name="w62", tag="wt")
            nc.scalar.dma_start(out=w62, in_=wp_d[:, w62_slots[0] * CO:])

            wtiles = []
            for c, (lo, hi) in enumerate(w_chunk_slots):
                t = wpool.tile([128, (hi - lo) * CO], wdt, name=f"wt{c}",
                               tag="wt")
                # alternate issue queues so per-DMA completion bookkeeping
                # (a ~1.7us engine slice) doesn't serialize the chunk stream
                eng = nc.sync if c % 2 == 0 else nc.scalar
                eng.dma_start(out=t, in_=wp_d[:, lo * CO:hi * CO])
                wtiles.append(t)
                if c == 0:
                    nc.sync.dma_start(out=xb, in_=xp_d[:, X_SPLIT * B:])
            wtiles.append(w62)
            w_chunk_slots.append(w62_slots)

            def xpair(q):                  # full [128, B] pair tile, q even
                j = q // 2
                if j < X_SPLIT:
                    return xa[:, j * B:(j + 1) * B]
                return xb[:, (j - X_SPLIT) * B:(j - X_SPLIT + 1) * B]

            outs = {}
            for oc, (lmax, groups) in enumerate(OUT_CHUNKS):
                outs[oc] = opool.tile([128, len(groups) * CO], bf16,
                                      name=f"ob{oc}", tag="ob")
            # l=63 does not exist: zero the never-written odd half of group
            # 31 so its out-DMA reads initialized memory.
            nc.vector.memset(outs[2][64:128, :], 0.0)

            ptile = [ppool.tile([128, CO], mybir.dt.float32, name=f"pb{t}")
                     for t in range(8)]

            # warm up the PE clock p-state while the first DMAs are in
            # flight: dummy matmuls on a zeroed tile into PSUM bank 7's
            # top half, which real work never touches (odd banks only ever
            # accumulate odd positions, i.e. partitions 64-127)
            warm = opool.tile([128, CO], bf16, name="warm")
            nc.vector.memset(warm[:, :], 0.0)
            for _ in range(24):
                nc.tensor.matmul(ptile[7][0:64, :], warm, warm,
                                 start=True, stop=True)

            group_chunk = {}
            for oc, (lmax, groups) in enumerate(OUT_CHUNKS):
                for gi, g in enumerate(groups):
                    group_chunk[g] = (oc, gi)

            # one PSUM bank per position pair (halves 0/1 = even/odd
            # position), so a single [128, 64] copy drains both positions
            bank_of_group = {}
            for seq, l in enumerate(g for g in ORDER if g % 2 == 0):
                bank_of_group[l // 2] = seq % 8

            for l in ORDER:
                c = chunk_of_pos[l]
                half = l % 2
                outp = ptile[bank_of_group[l // 2]][half * 64:half * 64 + 64, :]
                wt = wtiles[c]
                clo = w_chunk_slots[c][0]

                def rhs(s):
                    return wt[:, (slot_of(l, s) - clo) * CO:
                              (slot_of(l, s) - clo + 1) * CO]

                if half == 0:
                    for s in range(4):
                        nc.tensor.matmul(outp, xpair(l + 2 * s), rhs(s),
                                         start=(s == 0), stop=False)
                    nc.tensor.matmul(outp, xpair(l + 8)[0:64, :],
                                     rhs(4)[0:64, :], start=False, stop=True,
                                     tile_position=(0, half * 64))
                else:
                    nc.tensor.matmul(outp, xpair(l - 1)[64:128, :],
                                     rhs(4)[64:128, :], start=True, stop=False,
                                     tile_position=(64, half * 64))
                    for s in range(4):
                        nc.tensor.matmul(outp, xpair(l + 1 + 2 * s), rhs(s),
                                         start=False, stop=(s == 3))

                oc, gi = group_chunk[l // 2]
                # copy once per pair: both halves of the bank in one DVE op
                # (pos 62 has no odd partner; its copy is the top half only)
                if half == 1 or l == 62:
                    rows = slice(0, 64) if l == 62 else slice(0, 128)
                    nc.vector.tensor_copy(
                        out=outs[oc][rows, gi * CO:(gi + 1) * CO],
                        in_=ptile[bank_of_group[l // 2]][rows, :])
                for j, (lmax, groups) in enumerate(OUT_CHUNKS):
                    if l == lmax:
                        lo, hi = groups[0], groups[-1] + 1
                        # the tail-critical final chunk goes on the SP queue
                        # (idle by then); the rest on Activation
                        deng = nc.sync if j == len(OUT_CHUNKS) - 1 else nc.scalar
                        deng.dma_start(
                            out=out_d[:, lo * CO:hi * CO], in_=outs[j])
    _split_multi_waits(nc, mybir)
    return nc


def _split_multi_waits(nc, mybir):
    """This walrus build encodes at most ONE sync wait per instruction.

    Tile attaches multi-wait lists (e.g. on the kernel-tail Drain). Hoist
    all but the last wait onto single-wait NoOps inserted just before the
    instruction on the same engine -- semantically identical (the engine
    stalls at the NoOps instead of at the instruction itself).
    """
    for f in nc.m.functions:
        for bb in f.blocks:
            out = []
            for inst in bb.instructions:
                si = inst.sync_info
                waits = list(si.on_wait) if si is not None and si.on_wait else []
                if len(waits) > 1:
                    for k, w in enumerate(waits[:-1]):
                        out.append(mybir.InstNoOp(
                            name=f"{inst.name}-wsplit{k}",
                            engine=inst.engine,
                            sync_info=mybir.SyncInfo(on_wait=[w], on_update=[]),
                            bass_nofuse=True))
                    inst.sync_info = mybir.SyncInfo(
                        on_wait=[waits[-1]],
                        on_update=list(si.on_update) if si.on_update else [])
                out.append(inst)
            bb.instructions = out


def _prep_inputs(x, weight):
    """Returns list of 8 per-core input dicts."""
    wnp = (ml_dtypes.float8_e3m4 if WDT_NAME == "float8e3"
           else ml_dtypes.bfloat16)
    x = np.asarray(x, np.float32)
    w0 = np.asarray(weight, np.float32)[0]        # [CO, CI, L_OUT, K]

    wt = np.ascontiguousarray(w0.transpose(2, 3, 1, 0))   # [L_OUT, K, CI, CO]
    xt = x.transpose(1, 2, 0)                     # [CI, L, B]

    in_maps = []
    for m in range(N_CORES):
        hs = LP * m
        xh = np.zeros((CI, HALO + 1, B), np.float32)
        xh[:, :HALO] = xt[:, hs:hs + HALO]
        xp = np.zeros((128, NXBLK, B), np.float32)
        xp[:CI] = xh[:, 0::2]                      # even columns
        xp[CI:] = xh[:, 1::2]                      # odd columns

        wp = np.zeros((NSLOT, 128, CO), np.float32)
        for pair in range(NPAIR):
            le = hs + 2 * pair
            base = 9 * pair
            wp[base:base + 4] = wt[le, :8].reshape(4, 128, CO)
            wp[base + 4, :CI] = wt[le, 8]
            wp[base + 4, CI:] = wt[le + 1, 0]
            wp[base + 5:base + 9] = wt[le + 1, 1:9].reshape(4, 128, CO)
        base = 9 * NPAIR
        wp[base:base + 4] = wt[hs + LP - 1, :8].reshape(4, 128, CO)
        wp[base + 4, :CI] = wt[hs + LP - 1, 8]

        in_maps.append({
            "xp": np.ascontiguousarray(xp.reshape(128, NXBLK * B)).astype(
                ml_dtypes.bfloat16),
            "wp": np.ascontiguousarray(
                wp.transpose(1, 0, 2).reshape(128, NSLOT * CO)).astype(wnp),
        })
    return in_maps


def _decode_outputs(results):
    outs = []
    for r in results:
        v = np.asarray(r["out"]).astype(np.float32).reshape(2, 64, NGRP, CO)
        # v[h, b, g, o] -> out[b, o, l], l = 2g + h
        t = v.transpose(1, 3, 2, 0).reshape(B, CO, NGRP * 2)[:, :, :LP]
        outs.append(t)
    return np.concatenate(outs, axis=2)           # [B, CO, L_OUT]


_CACHED_NC = None


def kernel(x, weight):
    global _CACHED_NC
    from concourse.bass_utils import run_bass_kernel_spmd

    if _CACHED_NC is None:
        _CACHED_NC = _build_bass()
    in_maps = _prep_inputs(x, weight)
    res = run_bass_kernel_spmd(_CACHED_NC, in_maps, core_ids=list(range(N_CORES)))
    return _decode_outputs(res.results)
